# revision 4
# baseline (speedup 1.0000x reference)
"""Trainium2 Bass kernel for nn_CustomCNN (dense_cnn), v2.

Network (per image, 28x28 single channel):
  conv5x5(same) -> relu -> maxpool2     [28,28] -> [14,14]
  conv5x5(same) -> relu -> maxpool2     [14,14] -> [7,7]
  fc 49->128 + bias -> relu
  fc 128->10 + bias
  log_softmax

Strategy: pure data parallel over 8 NeuronCores (8192 images each).
v2 layout decisions (vs v1):
  - Host pre-transposes x to pixel-major tile form xt7 [7, 112, B] so the
    device does plain strided DMA loads (no DMA_TRANSPOSE on the sync queue).
  - Conv1 input tiles are chunk-aligned (rows 4m-2..4m+1), so every output
    chunk contracts exactly 2 tiles: 14 matmuls per 512-image group.
  - Pool1 = ACT relu-evict of the s=0 half (partition shift to base 0) +
    DVE max(SBUF, PSUM[64:]) (mixed-space TT allows different partition
    bases) + t-stage copy/max into a PACKED x2 layout [128, 2, 512] whose
    windows all start at 32-aligned partitions - no scatter DMAs.
  - Conv2 contracts the packed x2 in 2x2 matmuls; pool2 the same way.
"""

import os
import sys

import numpy as np

sys.path.insert(0, "/opt/trn_rl_repo")

import ml_dtypes

BF16 = ml_dtypes.bfloat16

# ---------------------------------------------------------------------------
# Problem constants (hardcoded per the harness contract)
# ---------------------------------------------------------------------------
B_TOTAL = 65536
N_CORES = 8
B_CORE = B_TOTAL // N_CORES          # 8192
NG = 512                             # images per group
N_GROUPS = B_CORE // NG              # 16
NSUB = NG // 128                     # 4 sub-chunks of 128 for fc2/log_softmax


# ---------------------------------------------------------------------------
# Host-side layout helpers
# ---------------------------------------------------------------------------

def _tile_home(p_flat):
    """Map input pixel p_flat (0..783) -> (col, row) in xt7 [7, 112, B].

    Tiles T_m = rows 4m-2..4m+1 (clipped): T_0 = px [0,56), T_m =
    [112m-56, 112m+56) for 1<=m<=6, T_7 = [728, 784).  Columns 0..5 hold
    T_1..T_6; column 6 holds [T_0 ; T_7]."""
    m = (p_flat + 56) // 112
    if m == 0:
        return 6, p_flat
    if m == 7:
        return 6, p_flat - 728 + 56
    return m - 1, p_flat - (112 * m - 56)


def _conv1_out_q(i, j, r):
    """Partition of conv1 output (i, j) in chunk r: q = s*64 + t*32 + a*14+u
    with i = 4r + 2a + s, j = 2u + t."""
    a, s = divmod(i - 4 * r, 2)
    u, t = divmod(j, 2)
    return s * 64 + t * 32 + a * 14 + u


def build_conv1_mats(k1):
    """wt1 [112, 14, 128]: slot 2r+jj contracts tile T_{r+jj} for chunk r."""
    mats = np.zeros((14, 112, 128), np.float32)
    for r in range(7):
        for i in range(4 * r, 4 * r + 4):
            for j in range(28):
                q = _conv1_out_q(i, j, r)
                for di in range(5):
                    for dj in range(5):
                        ii, jj = i + di - 2, j + dj - 2
                        if 0 <= ii < 28 and 0 <= jj < 28:
                            p_flat = 28 * ii + jj
                            m = (p_flat + 56) // 112
                            col, row = _tile_home(p_flat)
                            slot = 2 * r + (m - r)
                            assert m in (r, r + 1), (r, i, j, ii, jj, m)
                            mats[slot][row, q] += k1[di, dj]
    return np.ascontiguousarray(mats.transpose(1, 0, 2))   # [112, 14, 128]


def _x2_home(i2, j2):
    """Packed x2 home of pool1 output pixel (i2, j2), both 0..13.

    r = i2//2 (chunk), a = i2%2: col = r//4, partition 32*(r%4) + a*14 + j2."""
    r, a = divmod(i2, 2)
    return r // 4, 32 * (r % 4) + a * 14 + j2


def build_conv2_mats(k2):
    """wt2 [128, 4, 128]: slot s2*2+c contracts packed-x2 col c for out-row
    parity s2.  Out q2 = t2*64 + R*7 + u2 with i2o = 2R + s2, j2o = 2u2+t2."""
    mats = np.zeros((4, 128, 128), np.float32)
    for s2 in range(2):
        for R in range(7):
            i2o = 2 * R + s2
            for j2o in range(14):
                u2, t2 = divmod(j2o, 2)
                q2 = t2 * 64 + R * 7 + u2
                for di in range(5):
                    for dj in range(5):
                        i2, j2 = i2o + di - 2, j2o + dj - 2
                        if 0 <= i2 < 14 and 0 <= j2 < 14:
                            c, p2 = _x2_home(i2, j2)
                            mats[s2 * 2 + c][p2, q2] += k2[di, dj]
    return np.ascontiguousarray(mats.transpose(1, 0, 2))   # [128, 4, 128]


def build_host_weights(conv1_kernel, conv2_kernel, fc1_w, fc1_b, fc2_w, fc2_b):
    w1 = build_conv1_mats(np.asarray(conv1_kernel, np.float32))
    w2 = build_conv2_mats(np.asarray(conv2_kernel, np.float32))
    return {
        "wt1": w1.astype(BF16),                               # [112, 14, 128]
        "wt2": w2.astype(BF16),                               # [128, 4, 128]
        "fc1t": np.ascontiguousarray(np.asarray(fc1_w, np.float32).T).astype(BF16),  # [49, 128]
        "fc1b": np.asarray(fc1_b, np.float32).reshape(128, 1).copy(),
        "fc2t": np.ascontiguousarray(np.asarray(fc2_w, np.float32).T).astype(BF16),  # [128, 10]
        "fc2b": np.tile(np.asarray(fc2_b, np.float32).reshape(1, 10), (128, 1)),
    }


def build_xt7(xbf):
    """xbf [B, 784] bf16 -> xt7 [7, 112, B] bf16 (pixel-major tile form)."""
    B = xbf.shape[0]
    xt = np.ascontiguousarray(xbf.T)                         # [784, B]
    out = np.empty((7, 112, B), dtype=BF16)
    for m in range(1, 7):
        out[m - 1] = xt[112 * m - 56:112 * m + 56]
    out[6, 0:56] = xt[0:56]
    out[6, 56:112] = xt[728:784]
    return out


# ---------------------------------------------------------------------------
# Pure-numpy emulation of the device pipeline (layout validation / debug)
# ---------------------------------------------------------------------------

def emulate_pipeline(x, hw, n_images=512):
    """Exact device dataflow in numpy (bf16-rounded matmul inputs, fp32
    accumulation) for n_images. Returns [n_images, 10] float32."""
    w1 = hw["wt1"].astype(np.float32)        # [112, 14, 128]
    w2 = hw["wt2"].astype(np.float32)        # [128, 4, 128]
    xbf = np.asarray(x, np.float32).reshape(-1, 784)[:n_images].astype(BF16)
    xt7 = build_xt7(xbf).astype(np.float32)  # [7, 112, B]

    def col_for(slot):
        r, jj = divmod(slot, 2)
        m = r + jj
        return 6 if m in (0, 7) else m - 1

    # conv1: 7 chunk psums [128, B]
    x2 = np.zeros((128, 2, n_images), np.float32)
    for r in range(7):
        ps = np.zeros((128, n_images), np.float32)
        for jj in range(2):
            slot = 2 * r + jj
            ps += w1[:, slot, :].T @ xt7[col_for(slot)]
        a = np.maximum(ps[0:64], 0.0).astype(BF16).astype(np.float32)  # relu'd s0
        sm = np.maximum(a, ps[64:128]).astype(BF16).astype(np.float32)  # [64, B]
        tb = sm[32:64]
        tm = np.maximum(sm[0:32], tb)                                   # [32, B]
        x2[32 * (r % 4):32 * (r % 4) + 32, r // 4] = tm
    x2 = x2.astype(BF16).astype(np.float32)

    # conv2
    m2 = None
    ps2 = np.zeros((128, 2, n_images), np.float32)
    for s2 in range(2):
        for c in range(2):
            ps2[:, s2] += w2[:, s2 * 2 + c, :].T @ x2[:, c]
    c2a = np.maximum(ps2[:, 0], 0.0).astype(BF16).astype(np.float32)
    m2 = np.maximum(c2a, ps2[:, 1]).astype(BF16).astype(np.float32)
    x3 = np.maximum(m2[0:49], m2[64:113]).astype(BF16).astype(np.float32)

    f1 = hw["fc1t"].astype(np.float32).T @ x3 + hw["fc1b"]          # [128, B]
    h = np.maximum(f1, 0.0).astype(BF16).astype(np.float32)
    logits = (h.T @ hw["fc2t"].astype(np.float32)) + hw["fc2b"][0]  # [B, 10]
    mx = logits.max(1, keepdims=True)
    e = np.exp(logits - mx)
    return (logits - mx - np.log(e.sum(1, keepdims=True))).astype(np.float32)


# ---------------------------------------------------------------------------
# Bass kernel
# ---------------------------------------------------------------------------

def build_bass_kernel(n_groups=N_GROUPS):
    import concourse.bass as bass
    import concourse.tile as tile
    from concourse import bacc, mybir

    f32 = mybir.dt.float32
    bf16 = mybir.dt.bfloat16
    AF = mybir.ActivationFunctionType
    OP = mybir.AluOpType

    nc = bacc.Bacc("TRN2", target_bir_lowering=False, debug=False,
                   num_devices=N_CORES)

    b_core = n_groups * NG
    xt7 = nc.dram_tensor("xt7", [7, 112, b_core], bf16, kind="ExternalInput").ap()
    wt1 = nc.dram_tensor("wt1", [112, 14, 128], bf16, kind="ExternalInput").ap()
    wt2 = nc.dram_tensor("wt2", [128, 4, 128], bf16, kind="ExternalInput").ap()
    fc1t = nc.dram_tensor("fc1t", [49, 128], bf16, kind="ExternalInput").ap()
    fc1b = nc.dram_tensor("fc1b", [128, 1], f32, kind="ExternalInput").ap()
    fc2t = nc.dram_tensor("fc2t", [128, 10], bf16, kind="ExternalInput").ap()
    fc2b = nc.dram_tensor("fc2b", [128, 10], f32, kind="ExternalInput").ap()
    y = nc.dram_tensor("y", [b_core, 10], f32, kind="ExternalOutput").ap()

    def col_for(slot):
        r, jj = divmod(slot, 2)
        m = r + jj
        return 6 if m in (0, 7) else m - 1

    with tile.TileContext(nc) as tc:
        with (
            tc.tile_pool(name="wpool", bufs=1) as wpool,
            tc.tile_pool(name="inp", bufs=3) as inp,
            tc.tile_pool(name="work", bufs=3) as work,
            tc.tile_pool(name="hpool", bufs=6) as hpool,
            tc.tile_pool(name="outp", bufs=1) as outp,
            tc.tile_pool(name="psp", bufs=3, space="PSUM") as psp,
            tc.tile_pool(name="psl2", bufs=2, space="PSUM") as psl2,
        ):
            # ---- load weights once ----
            w1sb = wpool.tile([112, 14, 128], bf16)
            nc.sync.dma_start(w1sb, wt1)
            w2sb = wpool.tile([128, 4, 128], bf16)
            nc.sync.dma_start(w2sb, wt2)
            f1tsb = wpool.tile([49, 128], bf16)
            nc.sync.dma_start(f1tsb, fc1t)
            f1bsb = wpool.tile([128, 1], f32)
            nc.sync.dma_start(f1bsb, fc1b)
            f2tsb = wpool.tile([128, 10], bf16)
            nc.sync.dma_start(f2tsb, fc2t)
            f2bsb = wpool.tile([128, 10], f32)
            nc.sync.dma_start(f2bsb, fc2b)

            x2_all = outp.tile([128, n_groups, 2, NG], bf16)
            x3_all = outp.tile([49, n_groups, NG], bf16)

            # packed-x2 rows 96:128 of col 1 (r=7 slot) are never written;
            # conv2 weight rows there are zero, but the values must be finite.
            nc.gpsimd.memset(x2_all[96:128, :, 1, :], 0.0)

            xt7_v = xt7.rearrange("m p (g b) -> p m g b", g=n_groups)
            y_v = y.rearrange("(g u p) c -> p g u c", g=n_groups, u=NSUB)

            # ================= phase 1: conv1 + pool1 =================
            for g in range(n_groups):
                xp = inp.tile([112, 7, NG], bf16, tag="xp")
                nc.sync.dma_start(xp, xt7_v[:, :, g, :])

                # chunk pairs (0,1) (2,3) (4,5) and single (6,)
                for t in range(4):
                    rr = [2 * t, 2 * t + 1][:2 if t < 3 else 1]
                    w = len(rr)
                    pc = psp.tile([128, 2, NG], f32, tag="ps")
                    for i, r in enumerate(rr):
                        for jj in range(2):
                            slot = 2 * r + jj
                            nc.tensor.matmul(pc[:, i, :],
                                             w1sb[:, slot, :],
                                             xp[:, col_for(slot), :],
                                             start=(jj == 0), stop=(jj == 1))
                    # pool1: relu-evict s0 half, mixed-space s-max, t-stage
                    a_t = work.tile([64, 2, NG], bf16, tag="a")
                    nc.scalar.activation(a_t[:, 0:w, :], pc[0:64, 0:w, :],
                                         AF.Relu)
                    sm = work.tile([64, 2, NG], bf16, tag="sm")
                    nc.vector.tensor_tensor(sm[:, 0:w, :], a_t[:, 0:w, :],
                                            pc[64:128, 0:w, :], OP.max)
                    tb = work.tile([32, 2, NG], bf16, tag="tb")
                    nc.gpsimd.tensor_copy(tb[:, 0:w, :], sm[32:64, 0:w, :])
                    for i, r in enumerate(rr):
                        dst = x2_all[32 * (r % 4):32 * (r % 4) + 32,
                                     g, r // 4, :]
                        nc.vector.tensor_tensor(dst, sm[0:32, i, :],
                                                tb[:, i, :], OP.max)

            # ================= phase 2: conv2 + pool2 =================
            for g in range(n_groups):
                ps2 = psp.tile([128, 2, NG], f32, tag="ps")
                for s2 in range(2):
                    for c in range(2):
                        nc.tensor.matmul(ps2[:, s2, :],
                                         w2sb[:, s2 * 2 + c, :],
                                         x2_all[:, g, c, :],
                                         start=(c == 0), stop=(c == 1))
                c2a = work.tile([128, NG], bf16, tag="c2a")
                nc.scalar.activation(c2a, ps2[:, 0, :], AF.Relu)
                m2 = work.tile([128, NG], bf16, tag="m2")
                nc.vector.tensor_tensor(m2, c2a, ps2[:, 1, :], OP.max)
                m2b = work.tile([49, NG], bf16, tag="m2b")
                nc.gpsimd.tensor_copy(m2b, m2[64:113, :])
                nc.vector.tensor_tensor(x3_all[:, g, :], m2[0:49, :], m2b,
                                        OP.max)

            # ========== phase 3: fc1 + fc2 + log_softmax (4-group batches) ==
            for q in range(n_groups // 4):
                hs = []
                for j in range(4):
                    g = 4 * q + j
                    psf = psp.tile([128, NG], f32, tag="ps")
                    nc.tensor.matmul(psf, f1tsb, x3_all[:, g, :],
                                     start=True, stop=True)
                    h = hpool.tile([128, NG], bf16, tag="h")
                    nc.scalar.activation(h, psf, AF.Relu, bias=f1bsb[:, 0:1])
                    hs.append(h)

                psl = psl2.tile([128, 4, NSUB, 10], f32, tag="fc2")
                for j in range(4):
                    for u in range(NSUB):
                        nc.tensor.matmul(psl[:, j, u, :],
                                         hs[j][:, u * 128:(u + 1) * 128],
                                         f2tsb, start=True, stop=True)

                t1 = work.tile([128, 4, NSUB, 10], f32, tag="t1")
                nc.vector.tensor_tensor(
                    t1, psl,
                    f2bsb[:, None, None, :].to_broadcast((128, 4, NSUB, 10)),
                    OP.add)
                mx = work.tile([128, 4, NSUB], f32, tag="mx")
                nc.vector.tensor_reduce(mx, t1, mybir.AxisListType.X, OP.max)
                t2b = hpool.tile([128, 4, NSUB, 10], f32, tag="t2")
                nc.vector.tensor_tensor(
                    t2b, t1,
                    mx[:, :, :, None].to_broadcast((128, 4, NSUB, 10)),
                    OP.subtract)
                e = work.tile([128, 4, NSUB, 10], f32, tag="e")
                nc.scalar.activation(e, t2b, AF.Exp)
                ssum = work.tile([128, 4, NSUB], f32, tag="ssum")
                nc.vector.tensor_reduce(ssum, e, mybir.AxisListType.X, OP.add)
                lg = work.tile([128, 4, NSUB], f32, tag="lg")
                nc.scalar.activation(lg, ssum, AF.Ln)
                ob = hpool.tile([128, 4, NSUB, 10], f32, tag="ob")
                nc.vector.tensor_tensor(
                    ob, t2b,
                    lg[:, :, :, None].to_broadcast((128, 4, NSUB, 10)),
                    OP.subtract)
                nc.sync.dma_start(
                    y_v[:, 4 * q:4 * q + 4, :, :], ob)

    nc.compile()
    return nc


# ---------------------------------------------------------------------------
# Entry point
# ---------------------------------------------------------------------------

_CACHE = {}


def _install_ntff_hook():
    """Shim antenv.axon_hooks (absent on this image) with the ctypes hook
    from trn_agent_boot so run_bass_kernel_spmd(trace=True) can profile."""
    import types
    if "antenv.axon_hooks" in sys.modules:
        return
    try:
        from trn_agent_boot.trn_boot import _ntff_profile_via_ctypes
        hook = _ntff_profile_via_ctypes("/opt/axon/libaxon_pjrt.so")
    except Exception as e:
        print(f"ntff hook unavailable: {e}", file=sys.stderr)
        return
    if hook is None:
        return
    import antenv
    mod = types.ModuleType("antenv.axon_hooks")
    mod.get_axon_ntff_profile_hook = lambda: hook
    mod.set_axon_ntff_profile_hook = lambda h: None
    sys.modules["antenv.axon_hooks"] = mod
    antenv.axon_hooks = mod


def kernel(x, conv1_kernel, conv2_kernel, fc1_w, fc1_b, fc2_w, fc2_b):
    from concourse.bass_utils import run_bass_kernel_spmd

    hw = build_host_weights(conv1_kernel, conv2_kernel, fc1_w, fc1_b,
                            fc2_w, fc2_b)

    key = "nc"
    if key not in _CACHE:
        _CACHE[key] = build_bass_kernel()
    nc = _CACHE[key]

    xbf = np.asarray(x, np.float32).reshape(B_TOTAL, 784).astype(BF16)
    shared = {k: hw[k] for k in ("wt1", "wt2", "fc1t", "fc1b", "fc2t", "fc2b")}
    in_maps = []
    for c in range(N_CORES):
        m = dict(shared)
        m["xt7"] = build_xt7(xbf[c * B_CORE:(c + 1) * B_CORE])
        in_maps.append(m)

    trace = os.environ.get("KERNEL_TRACE", "0") == "1"
    if trace:
        _install_ntff_hook()
    res = run_bass_kernel_spmd(nc, in_maps, core_ids=list(range(N_CORES)),
                               trace=trace)
    if trace and res.exec_time_ns is not None:
        print(f"HW exec time: {res.exec_time_ns} ns", file=sys.stderr)
        _CACHE["exec_time_ns"] = res.exec_time_ns

    return np.concatenate([r["y"] for r in res.results], axis=0)


# revision 5
# speedup vs baseline: 1.9346x; 1.9346x over previous
"""Trainium2 Bass kernel for nn_CustomCNN (dense_cnn), v2.

Network (per image, 28x28 single channel):
  conv5x5(same) -> relu -> maxpool2     [28,28] -> [14,14]
  conv5x5(same) -> relu -> maxpool2     [14,14] -> [7,7]
  fc 49->128 + bias -> relu
  fc 128->10 + bias
  log_softmax

Strategy: pure data parallel over 8 NeuronCores (8192 images each).
v2 layout decisions (vs v1):
  - Host pre-transposes x to pixel-major tile form xt7 [7, 112, B] so the
    device does plain strided DMA loads (no DMA_TRANSPOSE on the sync queue).
  - Conv1 input tiles are chunk-aligned (rows 4m-2..4m+1), so every output
    chunk contracts exactly 2 tiles: 14 matmuls per 512-image group.
  - Pool1 = ACT relu-evict of the s=0 half (partition shift to base 0) +
    DVE max(SBUF, PSUM[64:]) (mixed-space TT allows different partition
    bases) + t-stage copy/max into a PACKED x2 layout [128, 2, 512] whose
    windows all start at 32-aligned partitions - no scatter DMAs.
  - Conv2 contracts the packed x2 in 2x2 matmuls; pool2 the same way.
"""

import os
import sys

import numpy as np

sys.path.insert(0, "/opt/trn_rl_repo")

import ml_dtypes

BF16 = ml_dtypes.bfloat16

# ---------------------------------------------------------------------------
# Problem constants (hardcoded per the harness contract)
# ---------------------------------------------------------------------------
B_TOTAL = 65536
N_CORES = 8
B_CORE = B_TOTAL // N_CORES          # 8192
NG = 512                             # images per group
N_GROUPS = B_CORE // NG              # 16
NSUB = NG // 128                     # 4 sub-chunks of 128 for fc2/log_softmax


# ---------------------------------------------------------------------------
# Host-side layout helpers
# ---------------------------------------------------------------------------

def _tile_home(p_flat):
    """Map input pixel p_flat (0..783) -> (col, row) in xt7 [7, 112, B].

    Tiles T_m = rows 4m-2..4m+1 (clipped): T_0 = px [0,56), T_m =
    [112m-56, 112m+56) for 1<=m<=6, T_7 = [728, 784).  Columns 0..5 hold
    T_1..T_6; column 6 holds [T_0 ; T_7]."""
    m = (p_flat + 56) // 112
    if m == 0:
        return 6, p_flat
    if m == 7:
        return 6, p_flat - 728 + 56
    return m - 1, p_flat - (112 * m - 56)


def _conv1_out_q(i, j, r):
    """Partition of conv1 output (i, j) in chunk r: q = s*64 + t*32 + a*14+u
    with i = 4r + 2a + s, j = 2u + t."""
    a, s = divmod(i - 4 * r, 2)
    u, t = divmod(j, 2)
    return s * 64 + t * 32 + a * 14 + u


def build_conv1_mats(k1):
    """wt1 [112, 14, 128]: slot 2r+jj contracts tile T_{r+jj} for chunk r."""
    mats = np.zeros((14, 112, 128), np.float32)
    for r in range(7):
        for i in range(4 * r, 4 * r + 4):
            for j in range(28):
                q = _conv1_out_q(i, j, r)
                for di in range(5):
                    for dj in range(5):
                        ii, jj = i + di - 2, j + dj - 2
                        if 0 <= ii < 28 and 0 <= jj < 28:
                            p_flat = 28 * ii + jj
                            m = (p_flat + 56) // 112
                            col, row = _tile_home(p_flat)
                            slot = 2 * r + (m - r)
                            assert m in (r, r + 1), (r, i, j, ii, jj, m)
                            mats[slot][row, q] += k1[di, dj]
    return np.ascontiguousarray(mats.transpose(1, 0, 2))   # [112, 14, 128]


def _x2_home(i2, j2):
    """Packed x2 home of pool1 output pixel (i2, j2), both 0..13.

    r = i2//2 (chunk), a = i2%2: col = r//4, partition 32*(r%4) + a*14 + j2."""
    r, a = divmod(i2, 2)
    return r // 4, 32 * (r % 4) + a * 14 + j2


def build_conv2_mats(k2):
    """wt2 [128, 4, 128]: slot s2*2+c contracts packed-x2 col c for out-row
    parity s2.  Out q2 = t2*64 + R*7 + u2 with i2o = 2R + s2, j2o = 2u2+t2."""
    mats = np.zeros((4, 128, 128), np.float32)
    for s2 in range(2):
        for R in range(7):
            i2o = 2 * R + s2
            for j2o in range(14):
                u2, t2 = divmod(j2o, 2)
                q2 = t2 * 64 + R * 7 + u2
                for di in range(5):
                    for dj in range(5):
                        i2, j2 = i2o + di - 2, j2o + dj - 2
                        if 0 <= i2 < 14 and 0 <= j2 < 14:
                            c, p2 = _x2_home(i2, j2)
                            mats[s2 * 2 + c][p2, q2] += k2[di, dj]
    return np.ascontiguousarray(mats.transpose(1, 0, 2))   # [128, 4, 128]


def build_host_weights(conv1_kernel, conv2_kernel, fc1_w, fc1_b, fc2_w, fc2_b):
    w1 = build_conv1_mats(np.asarray(conv1_kernel, np.float32))
    w2 = build_conv2_mats(np.asarray(conv2_kernel, np.float32))
    return {
        "wt1": w1.astype(BF16),                               # [112, 14, 128]
        "wt2": w2.astype(BF16),                               # [128, 4, 128]
        "fc1t": np.ascontiguousarray(np.asarray(fc1_w, np.float32).T).astype(BF16),  # [49, 128]
        "fc1b": np.asarray(fc1_b, np.float32).reshape(128, 1).copy(),
        "fc2t": np.ascontiguousarray(np.asarray(fc2_w, np.float32).T).astype(BF16),  # [128, 10]
        "fc2b": np.tile(np.asarray(fc2_b, np.float32).reshape(1, 10), (128, 1)),
    }


def build_xt7(xbf):
    """xbf [B, 784] bf16 -> xt7 [7, 112, B] bf16 (pixel-major tile form)."""
    B = xbf.shape[0]
    xt = np.ascontiguousarray(xbf.T)                         # [784, B]
    out = np.empty((7, 112, B), dtype=BF16)
    for m in range(1, 7):
        out[m - 1] = xt[112 * m - 56:112 * m + 56]
    out[6, 0:56] = xt[0:56]
    out[6, 56:112] = xt[728:784]
    return out


# ---------------------------------------------------------------------------
# Pure-numpy emulation of the device pipeline (layout validation / debug)
# ---------------------------------------------------------------------------

def emulate_pipeline(x, hw, n_images=512):
    """Exact device dataflow in numpy (bf16-rounded matmul inputs, fp32
    accumulation) for n_images. Returns [n_images, 10] float32."""
    w1 = hw["wt1"].astype(np.float32)        # [112, 14, 128]
    w2 = hw["wt2"].astype(np.float32)        # [128, 4, 128]
    xbf = np.asarray(x, np.float32).reshape(-1, 784)[:n_images].astype(BF16)
    xt7 = build_xt7(xbf).astype(np.float32)  # [7, 112, B]

    def col_for(slot):
        r, jj = divmod(slot, 2)
        m = r + jj
        return 6 if m in (0, 7) else m - 1

    # conv1: 7 chunk psums [128, B]
    x2 = np.zeros((128, 2, n_images), np.float32)
    for r in range(7):
        ps = np.zeros((128, n_images), np.float32)
        for jj in range(2):
            slot = 2 * r + jj
            ps += w1[:, slot, :].T @ xt7[col_for(slot)]
        a = np.maximum(ps[0:64], 0.0).astype(BF16).astype(np.float32)  # relu'd s0
        sm = np.maximum(a, ps[64:128]).astype(BF16).astype(np.float32)  # [64, B]
        tb = sm[32:64]
        tm = np.maximum(sm[0:32], tb)                                   # [32, B]
        x2[32 * (r % 4):32 * (r % 4) + 32, r // 4] = tm
    x2 = x2.astype(BF16).astype(np.float32)

    # conv2
    m2 = None
    ps2 = np.zeros((128, 2, n_images), np.float32)
    for s2 in range(2):
        for c in range(2):
            ps2[:, s2] += w2[:, s2 * 2 + c, :].T @ x2[:, c]
    c2a = np.maximum(ps2[:, 0], 0.0).astype(BF16).astype(np.float32)
    m2 = np.maximum(c2a, ps2[:, 1]).astype(BF16).astype(np.float32)
    x3 = np.maximum(m2[0:49], m2[64:113]).astype(BF16).astype(np.float32)

    f1 = hw["fc1t"].astype(np.float32).T @ x3 + hw["fc1b"]          # [128, B]
    h = np.maximum(f1, 0.0).astype(BF16).astype(np.float32)
    logits = (h.T @ hw["fc2t"].astype(np.float32)) + hw["fc2b"][0]  # [B, 10]
    mx = logits.max(1, keepdims=True)
    e = np.exp(logits - mx)
    return (logits - mx - np.log(e.sum(1, keepdims=True))).astype(np.float32)


# ---------------------------------------------------------------------------
# Bass kernel
# ---------------------------------------------------------------------------

def build_bass_kernel(n_groups=N_GROUPS):
    import concourse.bass as bass
    import concourse.tile as tile
    from concourse import bacc, mybir

    f32 = mybir.dt.float32
    bf16 = mybir.dt.bfloat16
    AF = mybir.ActivationFunctionType
    OP = mybir.AluOpType

    nc = bacc.Bacc("TRN2", target_bir_lowering=False, debug=False,
                   num_devices=N_CORES)

    b_core = n_groups * NG
    xt7 = nc.dram_tensor("xt7", [7, 112, b_core], bf16, kind="ExternalInput").ap()
    wt1 = nc.dram_tensor("wt1", [112, 14, 128], bf16, kind="ExternalInput").ap()
    wt2 = nc.dram_tensor("wt2", [128, 4, 128], bf16, kind="ExternalInput").ap()
    fc1t = nc.dram_tensor("fc1t", [49, 128], bf16, kind="ExternalInput").ap()
    fc1b = nc.dram_tensor("fc1b", [128, 1], f32, kind="ExternalInput").ap()
    fc2t = nc.dram_tensor("fc2t", [128, 10], bf16, kind="ExternalInput").ap()
    fc2b = nc.dram_tensor("fc2b", [128, 10], f32, kind="ExternalInput").ap()
    y = nc.dram_tensor("y", [128, n_groups, NSUB, 10], f32, kind="ExternalOutput").ap()

    def col_for(slot):
        r, jj = divmod(slot, 2)
        m = r + jj
        return 6 if m in (0, 7) else m - 1

    with tile.TileContext(nc) as tc:
        with (
            tc.tile_pool(name="wpool", bufs=1) as wpool,
            tc.tile_pool(name="inp", bufs=3) as inp,
            tc.tile_pool(name="work", bufs=3) as work,
            tc.tile_pool(name="hpool", bufs=6) as hpool,
            tc.tile_pool(name="outp", bufs=1) as outp,
            tc.tile_pool(name="psp", bufs=3, space="PSUM") as psp,
            tc.tile_pool(name="psl2", bufs=2, space="PSUM") as psl2,
        ):
            # ---- load weights once ----
            w1sb = wpool.tile([112, 14, 128], bf16)
            nc.sync.dma_start(w1sb, wt1)
            w2sb = wpool.tile([128, 4, 128], bf16)
            nc.sync.dma_start(w2sb, wt2)
            f1tsb = wpool.tile([49, 128], bf16)
            nc.sync.dma_start(f1tsb, fc1t)
            f1bsb = wpool.tile([128, 1], f32)
            nc.sync.dma_start(f1bsb, fc1b)
            f2tsb = wpool.tile([128, 10], bf16)
            nc.sync.dma_start(f2tsb, fc2t)
            f2bsb = wpool.tile([128, 10], f32)
            nc.sync.dma_start(f2bsb, fc2b)

            out_sb = outp.tile([128, n_groups, NSUB, 10], f32)
            t2_all = outp.tile([128, n_groups, NSUB, 10], f32)
            ssum_all = outp.tile([128, n_groups, NSUB], f32)
            x2_all = outp.tile([128, n_groups, 2, NG], bf16)
            x3_all = outp.tile([49, n_groups, NG], bf16)

            # packed-x2 rows 96:128 of col 1 (r=7 slot) are never written;
            # conv2 weight rows there are zero, but the values must be finite.
            nc.gpsimd.memset(x2_all[96:128, :, 1, :], 0.0)

            xt7_v = xt7.rearrange("m p (g b) -> p m g b", g=n_groups)

            # ================= phase 1: conv1 + pool1 =================
            for g in range(n_groups):
                xp = inp.tile([112, 7, NG], bf16, tag="xp")
                nc.sync.dma_start(xp, xt7_v[:, :, g, :])

                # chunk pairs (0,1) (2,3) (4,5) and single (6,)
                for t in range(4):
                    rr = [2 * t, 2 * t + 1][:2 if t < 3 else 1]
                    w = len(rr)
                    pc = psp.tile([128, 2, NG], f32, tag="ps")
                    for i, r in enumerate(rr):
                        for jj in range(2):
                            slot = 2 * r + jj
                            nc.tensor.matmul(pc[:, i, :],
                                             w1sb[:, slot, :],
                                             xp[:, col_for(slot), :],
                                             start=(jj == 0), stop=(jj == 1))
                    # pool1: relu-evict s0 half, mixed-space s-max, t-stage
                    a_t = work.tile([64, 2, NG], bf16, tag="a")
                    nc.scalar.activation(a_t[:, 0:w, :], pc[0:64, 0:w, :],
                                         AF.Relu)
                    sm = work.tile([64, 2, NG], bf16, tag="sm")
                    if t in (1, 2):
                        b_t = work.tile([64, 2, NG], bf16, tag="b")
                        nc.scalar.activation(b_t[:, 0:w, :], pc[64:128, 0:w, :],
                                             AF.Relu)
                        nc.vector.tensor_tensor(sm[:, 0:w, :], a_t[:, 0:w, :],
                                                b_t[:, 0:w, :], OP.max)
                    else:
                        nc.vector.tensor_tensor(sm[:, 0:w, :], a_t[:, 0:w, :],
                                                pc[64:128, 0:w, :], OP.max)
                    tb = work.tile([32, 2, NG], bf16, tag="tb")
                    nc.vector.tensor_copy(tb[:, 0:w, :], sm[32:64, 0:w, :])
                    for i, r in enumerate(rr):
                        dst = x2_all[32 * (r % 4):32 * (r % 4) + 32,
                                     g, r // 4, :]
                        nc.vector.tensor_tensor(dst, sm[0:32, i, :],
                                                tb[:, i, :], OP.max)

            # ================= phase 2: conv2 + pool2 =================
            for g in range(n_groups):
                ps2 = psp.tile([128, 2, NG], f32, tag="ps")
                for s2 in range(2):
                    for c in range(2):
                        nc.tensor.matmul(ps2[:, s2, :],
                                         w2sb[:, s2 * 2 + c, :],
                                         x2_all[:, g, c, :],
                                         start=(c == 0), stop=(c == 1))
                c2a = work.tile([128, NG], bf16, tag="c2a")
                nc.scalar.activation(c2a, ps2[:, 0, :], AF.Relu)
                m2 = work.tile([128, NG], bf16, tag="m2")
                nc.vector.tensor_tensor(m2, c2a, ps2[:, 1, :], OP.max)
                m2b = work.tile([49, NG], bf16, tag="m2b")
                nc.vector.tensor_copy(m2b, m2[64:113, :])
                nc.vector.tensor_tensor(x3_all[:, g, :], m2[0:49, :], m2b,
                                        OP.max)

            # ========== phase 3: fc1 + fc2 + log_softmax (4-group batches) ==
            for q in range(n_groups // 4):
                hs = []
                for j in range(4):
                    g = 4 * q + j
                    psf = psp.tile([128, NG], f32, tag="ps")
                    nc.tensor.matmul(psf, f1tsb, x3_all[:, g, :],
                                     start=True, stop=True)
                    h = hpool.tile([128, NG], bf16, tag="h")
                    nc.scalar.activation(h, psf, AF.Relu, bias=f1bsb[:, 0:1])
                    hs.append(h)

                psl = psl2.tile([128, 4, NSUB, 10], f32, tag="fc2")
                for j in range(4):
                    for u in range(NSUB):
                        nc.tensor.matmul(psl[:, j, u, :],
                                         hs[j][:, u * 128:(u + 1) * 128],
                                         f2tsb, start=True, stop=True)

                t1 = work.tile([128, 4, NSUB, 10], f32, tag="t1")
                nc.vector.tensor_tensor(
                    t1, psl,
                    f2bsb[:, None, None, :].to_broadcast((128, 4, NSUB, 10)),
                    OP.add)
                mx = work.tile([128, 4, NSUB], f32, tag="mx")
                nc.vector.tensor_reduce(mx, t1, mybir.AxisListType.X, OP.max)
                t2s = t2_all[:, 4 * q:4 * q + 4, :, :]
                nc.vector.tensor_tensor(
                    t2s, t1,
                    mx[:, :, :, None].to_broadcast((128, 4, NSUB, 10)),
                    OP.subtract)
                e = work.tile([128, 4, NSUB, 10], f32, tag="e")
                nc.scalar.activation(e, t2s, AF.Exp)
                nc.vector.tensor_reduce(ssum_all[:, 4 * q:4 * q + 4, :], e,
                                        mybir.AxisListType.X, OP.add)

            # ---- batched log + final subtract + store ----
            lg_all = outp.tile([128, n_groups, NSUB], f32)
            nc.scalar.activation(lg_all, ssum_all, AF.Ln)
            nc.vector.tensor_tensor(
                out_sb, t2_all,
                lg_all[:, :, :, None].to_broadcast((128, n_groups, NSUB, 10)),
                OP.subtract)
            nc.sync.dma_start(y, out_sb)

    nc.compile()
    return nc


# ---------------------------------------------------------------------------
# Entry point
# ---------------------------------------------------------------------------

_CACHE = {}


def _install_ntff_hook():
    """Shim antenv.axon_hooks (absent on this image) with the ctypes hook
    from trn_agent_boot so run_bass_kernel_spmd(trace=True) can profile."""
    import types
    if "antenv.axon_hooks" in sys.modules:
        return
    try:
        from trn_agent_boot.trn_boot import _ntff_profile_via_ctypes
        hook = _ntff_profile_via_ctypes("/opt/axon/libaxon_pjrt.so")
    except Exception as e:
        print(f"ntff hook unavailable: {e}", file=sys.stderr)
        return
    if hook is None:
        return
    import antenv
    mod = types.ModuleType("antenv.axon_hooks")
    mod.get_axon_ntff_profile_hook = lambda: hook
    mod.set_axon_ntff_profile_hook = lambda h: None
    sys.modules["antenv.axon_hooks"] = mod
    antenv.axon_hooks = mod


def kernel(x, conv1_kernel, conv2_kernel, fc1_w, fc1_b, fc2_w, fc2_b):
    from concourse.bass_utils import run_bass_kernel_spmd

    hw = build_host_weights(conv1_kernel, conv2_kernel, fc1_w, fc1_b,
                            fc2_w, fc2_b)

    key = "nc"
    if key not in _CACHE:
        _CACHE[key] = build_bass_kernel()
    nc = _CACHE[key]

    xbf = np.asarray(x, np.float32).reshape(B_TOTAL, 784).astype(BF16)
    shared = {k: hw[k] for k in ("wt1", "wt2", "fc1t", "fc1b", "fc2t", "fc2b")}
    in_maps = []
    for c in range(N_CORES):
        m = dict(shared)
        m["xt7"] = build_xt7(xbf[c * B_CORE:(c + 1) * B_CORE])
        in_maps.append(m)

    trace = os.environ.get("KERNEL_TRACE", "0") == "1"
    if trace:
        _install_ntff_hook()
    res = run_bass_kernel_spmd(nc, in_maps, core_ids=list(range(N_CORES)),
                               trace=trace)
    if trace and res.exec_time_ns is not None:
        print(f"HW exec time: {res.exec_time_ns} ns", file=sys.stderr)
        _CACHE["exec_time_ns"] = res.exec_time_ns

    outs = []
    for r in res.results:
        yc = r["y"]                          # [128, n_groups, NSUB, 10]
        outs.append(np.ascontiguousarray(
            yc.transpose(1, 2, 0, 3).reshape(B_CORE, 10)))
    return np.concatenate(outs, axis=0)


# revision 6
# speedup vs baseline: 2.0371x; 1.0530x over previous
"""Trainium2 Bass kernel for nn_CustomCNN (dense_cnn), v2.

Network (per image, 28x28 single channel):
  conv5x5(same) -> relu -> maxpool2     [28,28] -> [14,14]
  conv5x5(same) -> relu -> maxpool2     [14,14] -> [7,7]
  fc 49->128 + bias -> relu
  fc 128->10 + bias
  log_softmax

Strategy: pure data parallel over 8 NeuronCores (8192 images each).
v2 layout decisions (vs v1):
  - Host pre-transposes x to pixel-major tile form xt7 [7, 112, B] so the
    device does plain strided DMA loads (no DMA_TRANSPOSE on the sync queue).
  - Conv1 input tiles are chunk-aligned (rows 4m-2..4m+1), so every output
    chunk contracts exactly 2 tiles: 14 matmuls per 512-image group.
  - Pool1 = ACT relu-evict of the s=0 half (partition shift to base 0) +
    DVE max(SBUF, PSUM[64:]) (mixed-space TT allows different partition
    bases) + t-stage copy/max into a PACKED x2 layout [128, 2, 512] whose
    windows all start at 32-aligned partitions - no scatter DMAs.
  - Conv2 contracts the packed x2 in 2x2 matmuls; pool2 the same way.
"""

import os
import sys

import numpy as np

sys.path.insert(0, "/opt/trn_rl_repo")

import ml_dtypes

BF16 = ml_dtypes.bfloat16

# ---------------------------------------------------------------------------
# Problem constants (hardcoded per the harness contract)
# ---------------------------------------------------------------------------
B_TOTAL = 65536
N_CORES = 8
B_CORE = B_TOTAL // N_CORES          # 8192
NG = 512                             # images per group
N_GROUPS = B_CORE // NG              # 16
NSUB = NG // 128                     # 4 sub-chunks of 128 for fc2/log_softmax


# ---------------------------------------------------------------------------
# Host-side layout helpers
# ---------------------------------------------------------------------------

def _tile_home(p_flat):
    """Map input pixel p_flat (0..783) -> (col, row) in xt7 [7, 112, B].

    Tiles T_m = rows 4m-2..4m+1 (clipped): T_0 = px [0,56), T_m =
    [112m-56, 112m+56) for 1<=m<=6, T_7 = [728, 784).  Columns 0..5 hold
    T_1..T_6; column 6 holds [T_0 ; T_7]."""
    m = (p_flat + 56) // 112
    if m == 0:
        return 6, p_flat
    if m == 7:
        return 6, p_flat - 728 + 56
    return m - 1, p_flat - (112 * m - 56)


def _conv1_out_q(i, j, r):
    """Partition of conv1 output (i, j) in chunk r: q = s*64 + t*32 + a*14+u
    with i = 4r + 2a + s, j = 2u + t."""
    a, s = divmod(i - 4 * r, 2)
    u, t = divmod(j, 2)
    return s * 64 + t * 32 + a * 14 + u


def build_conv1_mats(k1):
    """wt1 [112, 14, 128]: slot 2r+jj contracts tile T_{r+jj} for chunk r."""
    mats = np.zeros((14, 128, 128), np.float32)
    for r in range(7):
        for i in range(4 * r, 4 * r + 4):
            for j in range(28):
                q = _conv1_out_q(i, j, r)
                for di in range(5):
                    for dj in range(5):
                        ii, jj = i + di - 2, j + dj - 2
                        if 0 <= ii < 28 and 0 <= jj < 28:
                            p_flat = 28 * ii + jj
                            m = (p_flat + 56) // 112
                            col, row = _tile_home(p_flat)
                            slot = 2 * r + (m - r)
                            assert m in (r, r + 1), (r, i, j, ii, jj, m)
                            mats[slot][row, q] += k1[di, dj]
    return np.ascontiguousarray(mats.transpose(1, 0, 2))   # [128, 14, 128]


def _x2_home(i2, j2):
    """Packed x2 home of pool1 output pixel (i2, j2), both 0..13.

    r = i2//2 (chunk), a = i2%2: col = r//4, partition 32*(r%4) + a*14 + j2."""
    r, a = divmod(i2, 2)
    return r // 4, 32 * (r % 4) + a * 14 + j2


def build_conv2_mats(k2):
    """wt2 [128, 4, 128]: slot s2*2+c contracts packed-x2 col c for out-row
    parity s2.  Out q2 = t2*64 + R*7 + u2 with i2o = 2R + s2, j2o = 2u2+t2."""
    mats = np.zeros((4, 128, 128), np.float32)
    for s2 in range(2):
        for R in range(7):
            i2o = 2 * R + s2
            for j2o in range(14):
                u2, t2 = divmod(j2o, 2)
                q2 = t2 * 64 + R * 7 + u2
                for di in range(5):
                    for dj in range(5):
                        i2, j2 = i2o + di - 2, j2o + dj - 2
                        if 0 <= i2 < 14 and 0 <= j2 < 14:
                            c, p2 = _x2_home(i2, j2)
                            mats[s2 * 2 + c][p2, q2] += k2[di, dj]
    return np.ascontiguousarray(mats.transpose(1, 0, 2))   # [128, 4, 128]


def build_host_weights(conv1_kernel, conv2_kernel, fc1_w, fc1_b, fc2_w, fc2_b):
    w1 = build_conv1_mats(np.asarray(conv1_kernel, np.float32))
    w2 = build_conv2_mats(np.asarray(conv2_kernel, np.float32))
    return {
        "wt1": w1.astype(BF16),                               # [112, 14, 128]
        "wt2": w2.astype(BF16),                               # [128, 4, 128]
        "fc1t": np.ascontiguousarray(np.asarray(fc1_w, np.float32).T).astype(BF16),  # [49, 128]
        "fc1b": np.asarray(fc1_b, np.float32).reshape(128, 1).copy(),
        "fc2t": np.ascontiguousarray(np.asarray(fc2_w, np.float32).T).astype(BF16),  # [128, 10]
        "fc2b": np.tile(np.asarray(fc2_b, np.float32).reshape(1, 10), (128, 1)),
    }


def build_xt7(xbf):
    """xbf [B, 784] bf16 -> xt7 [7, 128, B] bf16 (pixel-major tile form,
    rows 112:128 zero-padded so conv1 lhsT is full 128 rows -> FWL)."""
    B = xbf.shape[0]
    xt = np.ascontiguousarray(xbf.T)                         # [784, B]
    out = np.zeros((7, 128, B), dtype=BF16)
    for m in range(1, 7):
        out[m - 1, 0:112] = xt[112 * m - 56:112 * m + 56]
    out[6, 0:56] = xt[0:56]
    out[6, 56:112] = xt[728:784]
    return out


# ---------------------------------------------------------------------------
# Pure-numpy emulation of the device pipeline (layout validation / debug)
# ---------------------------------------------------------------------------

def emulate_pipeline(x, hw, n_images=512):
    """Exact device dataflow in numpy (bf16-rounded matmul inputs, fp32
    accumulation) for n_images. Returns [n_images, 10] float32."""
    w1 = hw["wt1"].astype(np.float32)        # [128, 14, 128]
    w2 = hw["wt2"].astype(np.float32)        # [128, 4, 128]
    xbf = np.asarray(x, np.float32).reshape(-1, 784)[:n_images].astype(BF16)
    xt7 = build_xt7(xbf).astype(np.float32)  # [7, 112, B]

    def col_for(slot):
        r, jj = divmod(slot, 2)
        m = r + jj
        return 6 if m in (0, 7) else m - 1

    # conv1: 7 chunk psums [128, B]
    x2 = np.zeros((128, 2, n_images), np.float32)
    for r in range(7):
        ps = np.zeros((128, n_images), np.float32)
        for jj in range(2):
            slot = 2 * r + jj
            ps += w1[:, slot, :].T @ xt7[col_for(slot)]
        a = np.maximum(ps[0:64], 0.0).astype(BF16).astype(np.float32)  # relu'd s0
        sm = np.maximum(a, ps[64:128]).astype(BF16).astype(np.float32)  # [64, B]
        tb = sm[32:64]
        tm = np.maximum(sm[0:32], tb)                                   # [32, B]
        x2[32 * (r % 4):32 * (r % 4) + 32, r // 4] = tm
    x2 = x2.astype(BF16).astype(np.float32)

    # conv2
    m2 = None
    ps2 = np.zeros((128, 2, n_images), np.float32)
    for s2 in range(2):
        for c in range(2):
            ps2[:, s2] += w2[:, s2 * 2 + c, :].T @ x2[:, c]
    c2a = np.maximum(ps2[:, 0], 0.0).astype(BF16).astype(np.float32)
    m2 = np.maximum(c2a, ps2[:, 1]).astype(BF16).astype(np.float32)
    x3 = np.maximum(m2[0:49], m2[64:113]).astype(BF16).astype(np.float32)

    f1 = hw["fc1t"].astype(np.float32).T @ x3 + hw["fc1b"]          # [128, B]
    h = np.maximum(f1, 0.0).astype(BF16).astype(np.float32)
    logits = (h.T @ hw["fc2t"].astype(np.float32)) + hw["fc2b"][0]  # [B, 10]
    e = np.exp(logits)
    return (logits - np.log(e.sum(1, keepdims=True))).astype(np.float32)


# ---------------------------------------------------------------------------
# Bass kernel
# ---------------------------------------------------------------------------

def build_bass_kernel(n_groups=N_GROUPS):
    import concourse.bass as bass
    import concourse.tile as tile
    from concourse import bacc, mybir

    f32 = mybir.dt.float32
    bf16 = mybir.dt.bfloat16
    AF = mybir.ActivationFunctionType
    OP = mybir.AluOpType

    nc = bacc.Bacc("TRN2", target_bir_lowering=False, debug=False,
                   num_devices=N_CORES)

    b_core = n_groups * NG
    xt7 = nc.dram_tensor("xt7", [7, 128, b_core], bf16, kind="ExternalInput").ap()
    wt1 = nc.dram_tensor("wt1", [128, 14, 128], bf16, kind="ExternalInput").ap()
    wt2 = nc.dram_tensor("wt2", [128, 4, 128], bf16, kind="ExternalInput").ap()
    fc1t = nc.dram_tensor("fc1t", [49, 128], bf16, kind="ExternalInput").ap()
    fc1b = nc.dram_tensor("fc1b", [128, 1], f32, kind="ExternalInput").ap()
    fc2t = nc.dram_tensor("fc2t", [128, 10], bf16, kind="ExternalInput").ap()
    fc2b = nc.dram_tensor("fc2b", [128, 10], f32, kind="ExternalInput").ap()
    y = nc.dram_tensor("y", [128, n_groups, NSUB, 10], f32, kind="ExternalOutput").ap()

    def col_for(slot):
        r, jj = divmod(slot, 2)
        m = r + jj
        return 6 if m in (0, 7) else m - 1

    with tile.TileContext(nc) as tc:
        with (
            tc.tile_pool(name="wpool", bufs=1) as wpool,
            tc.tile_pool(name="inp", bufs=3) as inp,
            tc.tile_pool(name="work", bufs=3) as work,
            tc.tile_pool(name="hpool", bufs=6) as hpool,
            tc.tile_pool(name="outp", bufs=1) as outp,
            tc.tile_pool(name="psp", bufs=3, space="PSUM") as psp,
            tc.tile_pool(name="psl2", bufs=2, space="PSUM") as psl2,
        ):
            # ---- load weights once ----
            w1sb = wpool.tile([128, 14, 128], bf16)
            nc.sync.dma_start(w1sb, wt1)
            w2sb = wpool.tile([128, 4, 128], bf16)
            nc.sync.dma_start(w2sb, wt2)
            f1tsb = wpool.tile([49, 128], bf16)
            nc.sync.dma_start(f1tsb, fc1t)
            f1bsb = wpool.tile([128, 1], f32)
            nc.sync.dma_start(f1bsb, fc1b)
            f2tsb = wpool.tile([128, 10], bf16)
            nc.sync.dma_start(f2tsb, fc2t)
            f2bsb = wpool.tile([128, 10], f32)
            nc.sync.dma_start(f2bsb, fc2b)

            out_sb = outp.tile([128, n_groups, NSUB, 10], f32)
            t2_all = outp.tile([128, n_groups, NSUB, 10], f32)
            ssum_all = outp.tile([128, n_groups, NSUB], f32)
            x2_all = outp.tile([128, n_groups, 2, NG], bf16)
            x3_all = outp.tile([49, n_groups, NG], bf16)

            # packed-x2 rows 96:128 of col 1 (r=7 slot) are never written;
            # conv2 weight rows there are zero, but the values must be finite.
            nc.gpsimd.memset(x2_all[96:128, :, 1, :], 0.0)

            xt7_v = xt7.rearrange("m p (g b) -> p m g b", g=n_groups)

            # ================= phase 1: conv1 + pool1 =================
            for g in range(n_groups):
                xp = inp.tile([128, 7, NG], bf16, tag="xp")
                nc.sync.dma_start(xp, xt7_v[:, :, g, :])

                # chunk pairs (0,1) (2,3) (4,5) and single (6,)
                for t in range(4):
                    rr = [2 * t, 2 * t + 1][:2 if t < 3 else 1]
                    w = len(rr)
                    pc = psp.tile([128, 2, NG], f32, tag="ps")
                    for i, r in enumerate(rr):
                        for jj in range(2):
                            slot = 2 * r + jj
                            nc.tensor.matmul(pc[:, i, :],
                                             w1sb[:, slot, :],
                                             xp[:, col_for(slot), :],
                                             start=(jj == 0), stop=(jj == 1))
                    # pool1: relu-evict s0 half, mixed-space s-max, t-stage
                    a_t = work.tile([64, 2, NG], bf16, tag="a")
                    nc.scalar.activation(a_t[:, 0:w, :], pc[0:64, 0:w, :],
                                         AF.Relu)
                    sm = work.tile([64, 2, NG], bf16, tag="sm")
                    if t in (0, 1, 2):
                        b_t = work.tile([64, 2, NG], bf16, tag="b")
                        nc.scalar.activation(b_t[:, 0:w, :], pc[64:128, 0:w, :],
                                             AF.Relu)
                        nc.vector.tensor_tensor(sm[:, 0:w, :], a_t[:, 0:w, :],
                                                b_t[:, 0:w, :], OP.max)
                    else:
                        nc.vector.tensor_tensor(sm[:, 0:w, :], a_t[:, 0:w, :],
                                                pc[64:128, 0:w, :], OP.max)
                    tb = work.tile([32, 2, NG], bf16, tag="tb")
                    nc.vector.tensor_copy(tb[:, 0:w, :], sm[32:64, 0:w, :])
                    for i, r in enumerate(rr):
                        dst = x2_all[32 * (r % 4):32 * (r % 4) + 32,
                                     g, r // 4, :]
                        nc.vector.tensor_tensor(dst, sm[0:32, i, :],
                                                tb[:, i, :], OP.max)

            # ================= phase 2: conv2 + pool2 =================
            for g in range(n_groups):
                ps2 = psp.tile([128, 2, NG], f32, tag="ps")
                for s2 in range(2):
                    for c in range(2):
                        nc.tensor.matmul(ps2[:, s2, :],
                                         w2sb[:, s2 * 2 + c, :],
                                         x2_all[:, g, c, :],
                                         start=(c == 0), stop=(c == 1))
                c2a = work.tile([128, NG], bf16, tag="c2a")
                nc.scalar.activation(c2a, ps2[:, 0, :], AF.Relu)
                m2 = work.tile([128, NG], bf16, tag="m2")
                nc.vector.tensor_tensor(m2, c2a, ps2[:, 1, :], OP.max)
                m2b = work.tile([49, NG], bf16, tag="m2b")
                nc.vector.tensor_copy(m2b, m2[64:113, :])
                nc.vector.tensor_tensor(x3_all[:, g, :], m2[0:49, :], m2b,
                                        OP.max)

            # ========== phase 3: fc1 + fc2 + log_softmax (4-group batches) ==
            for q in range(n_groups // 4):
                hs = []
                for j in range(4):
                    g = 4 * q + j
                    psf = psp.tile([128, NG], f32, tag="ps")
                    nc.tensor.matmul(psf, f1tsb, x3_all[:, g, :],
                                     start=True, stop=True)
                    h = hpool.tile([128, NG], bf16, tag="h")
                    nc.scalar.activation(h, psf, AF.Relu, bias=f1bsb[:, 0:1])
                    hs.append(h)

                psl = psl2.tile([128, 4, NSUB, 10], f32, tag="fc2")
                for j in range(4):
                    for u in range(NSUB):
                        nc.tensor.matmul(psl[:, j, u, :],
                                         hs[j][:, u * 128:(u + 1) * 128],
                                         f2tsb, start=True, stop=True)

                t2s = t2_all[:, 4 * q:4 * q + 4, :, :]
                nc.vector.tensor_tensor(
                    t2s, psl,
                    f2bsb[:, None, None, :].to_broadcast((128, 4, NSUB, 10)),
                    OP.add)
                e = work.tile([128, 4, NSUB, 10], f32, tag="e")
                nc.scalar.activation(e, t2s, AF.Exp)
                nc.vector.tensor_reduce(ssum_all[:, 4 * q:4 * q + 4, :], e,
                                        mybir.AxisListType.X, OP.add)

            # ---- batched log + final subtract + store ----
            lg_all = outp.tile([128, n_groups, NSUB], f32)
            nc.scalar.activation(lg_all, ssum_all, AF.Ln)
            nc.vector.tensor_tensor(
                out_sb, t2_all,
                lg_all[:, :, :, None].to_broadcast((128, n_groups, NSUB, 10)),
                OP.subtract)
            nc.sync.dma_start(y, out_sb)

    nc.compile()
    return nc


# ---------------------------------------------------------------------------
# Entry point
# ---------------------------------------------------------------------------

_CACHE = {}


def _install_ntff_hook():
    """Shim antenv.axon_hooks (absent on this image) with the ctypes hook
    from trn_agent_boot so run_bass_kernel_spmd(trace=True) can profile."""
    import types
    if "antenv.axon_hooks" in sys.modules:
        return
    try:
        from trn_agent_boot.trn_boot import _ntff_profile_via_ctypes
        hook = _ntff_profile_via_ctypes("/opt/axon/libaxon_pjrt.so")
    except Exception as e:
        print(f"ntff hook unavailable: {e}", file=sys.stderr)
        return
    if hook is None:
        return
    import antenv
    mod = types.ModuleType("antenv.axon_hooks")
    mod.get_axon_ntff_profile_hook = lambda: hook
    mod.set_axon_ntff_profile_hook = lambda h: None
    sys.modules["antenv.axon_hooks"] = mod
    antenv.axon_hooks = mod


def kernel(x, conv1_kernel, conv2_kernel, fc1_w, fc1_b, fc2_w, fc2_b):
    from concourse.bass_utils import run_bass_kernel_spmd

    hw = build_host_weights(conv1_kernel, conv2_kernel, fc1_w, fc1_b,
                            fc2_w, fc2_b)

    key = "nc"
    if key not in _CACHE:
        _CACHE[key] = build_bass_kernel()
    nc = _CACHE[key]

    xbf = np.asarray(x, np.float32).reshape(B_TOTAL, 784).astype(BF16)
    shared = {k: hw[k] for k in ("wt1", "wt2", "fc1t", "fc1b", "fc2t", "fc2b")}
    in_maps = []
    for c in range(N_CORES):
        m = dict(shared)
        m["xt7"] = build_xt7(xbf[c * B_CORE:(c + 1) * B_CORE])
        in_maps.append(m)

    trace = os.environ.get("KERNEL_TRACE", "0") == "1"
    if trace:
        _install_ntff_hook()
    res = run_bass_kernel_spmd(nc, in_maps, core_ids=list(range(N_CORES)),
                               trace=trace)
    if trace and res.exec_time_ns is not None:
        print(f"HW exec time: {res.exec_time_ns} ns", file=sys.stderr)
        _CACHE["exec_time_ns"] = res.exec_time_ns

    outs = []
    for r in res.results:
        yc = r["y"]                          # [128, n_groups, NSUB, 10]
        outs.append(np.ascontiguousarray(
            yc.transpose(1, 2, 0, 3).reshape(B_CORE, 10)))
    return np.concatenate(outs, axis=0)
